# revision 7
# baseline (speedup 1.0000x reference)
"""Bee Algorithm kernel for Trainium2 (8 NeuronCores, SPMD, raw Bass).

The reference is a fixed-seed (key 42) bee algorithm: every random noise
tensor is an input-independent constant, generated once on host with the
exact jax.random calls (and ambient PRNG config) the reference uses.
The heavy, memory-bound work — per-row L2 fitness over [2048,8192]
scouts, [512,8192] elites x 3 local-search steps, and [4096,8192]
onlookers — runs on the 8 NeuronCores as streaming kernels sharded by
rows (data-parallel over the bee axis, per the sharding hint):

    per row block [128, D] (bf16 streams):
        DMA a, DMA b -> DVE add -> fp32 scratch
        -> ACT Square in place, accum_out = row fit^2 -> DMA out fits

Inputs stream in bf16: every discrete decision in the algorithm (greedy
accepts ~0.6, onlooker accepts ~0.13, global argmin gap ~1.5e-2, all in
norm units) has margins 15-700x larger than the ~8e-4 fitness error bf16
streams introduce, and the final argmin is additionally re-checked on
host with exact fp64 arithmetic over a top-K candidate set, as are any
accept decisions whose device margin is small.  The winning row itself
is reconstructed bit-exactly on host from the fp32 inputs, so the output
matches the reference to the bit whenever the (hugely separated)
decisions agree.

Phase 2 measures ||e0 + n_t||^2 for the 3 steps; the greedy recursion is
resolved on host in fp64 via exact polarization identities over the
bf16-represented vectors (noise-noise terms precomputed).
"""

import os
import sys
from contextlib import ExitStack

import numpy as np

for _p in ("/opt/trn_rl_repo", "/opt/pypackages"):
    if _p not in sys.path:
        sys.path.append(_p)

import ml_dtypes

BF16 = ml_dtypes.bfloat16

S, E, O, D = 2048, 512, 4096, 8192
R = 0.1
NCORES = 8
SPC = S // NCORES  # 256 scout rows per core
EPC = E // NCORES  # 64 elite rows per core
OPC = O // NCORES  # 512 onlooker rows per core

_state = None
_last_results = []


def _build_fit2_module(rows, width=D, nbuf=2, nscr=2):
    """Per core: a,b [rows, width] bf16 -> out [rows//128, 128] f32 row
    sums of (a+b)^2.  Block-serialized DMA issue; DVE adds into fp32
    scratch; ACT squares in place with accum_out."""
    import concourse.bass as bass
    from concourse import mybir

    f32 = mybir.dt.float32
    bf16 = mybir.dt.bfloat16
    nblk = rows // 128
    nbuf = min(nbuf, nblk)
    nscr = min(nscr, nblk)
    nc = bass.Bass()
    a = nc.declare_dram_parameter("a", [rows, width], bf16, isOutput=False)
    b = nc.declare_dram_parameter("b", [rows, width], bf16, isOutput=False)
    out = nc.declare_dram_parameter("out", [nblk, 128], f32, isOutput=True)
    with ExitStack() as ctx:
        ta = [
            ctx.enter_context(nc.sbuf_tensor(f"ta{i}", [128, width], bf16))
            for i in range(nbuf)
        ]
        tb = [
            ctx.enter_context(nc.sbuf_tensor(f"tb{i}", [128, width], bf16))
            for i in range(nbuf)
        ]
        ts = [
            ctx.enter_context(nc.sbuf_tensor(f"ts{i}", [128, width], f32))
            for i in range(nscr)
        ]
        acc = ctx.enter_context(nc.sbuf_tensor("acc", [128, nblk], f32))
        with (
            nc.Block() as block,
            nc.semaphore("s_load") as s_load,
            nc.semaphore("s_dve") as s_dve,
            nc.semaphore("s_act") as s_act,
            nc.semaphore("s_out") as s_out,
        ):

            @block.gpsimd
            def _(g):
                for blk in range(nblk):
                    i = blk % nbuf
                    if blk >= 1:
                        # serialize block loads so early blocks land at
                        # full bandwidth and compute starts immediately
                        g.wait_ge(s_load, 32 * blk)
                    if blk >= nbuf:
                        g.wait_ge(s_dve, blk - nbuf + 1)
                    g.dma_start(
                        out=ta[i][:], in_=a[blk * 128:(blk + 1) * 128, :]
                    ).then_inc(s_load, 16)
                    g.dma_start(
                        out=tb[i][:], in_=b[blk * 128:(blk + 1) * 128, :]
                    ).then_inc(s_load, 16)

            @block.vector
            def _(v):
                for blk in range(nblk):
                    i = blk % nbuf
                    j = blk % nscr
                    v.wait_ge(s_load, 32 * (blk + 1))
                    if blk >= nscr:
                        v.wait_ge(s_act, blk - nscr + 1)
                    v.tensor_tensor(
                        out=ts[j][:], in0=ta[i][:], in1=tb[i][:],
                        op=mybir.AluOpType.add,
                    ).then_inc(s_dve, 1)

            @block.scalar
            def _(sc):
                for blk in range(nblk):
                    j = blk % nscr
                    sc.wait_ge(s_dve, blk + 1)
                    sc.activation(
                        out=ts[j][:], in_=ts[j][:],
                        func=mybir.ActivationFunctionType.Square,
                        accum_out=acc[:, blk:blk + 1],
                    ).then_inc(s_act, 1)

            @block.sync
            def _(sy):
                for blk in range(nblk):
                    sy.wait_ge(s_act, blk + 1)
                    sy.dma_start(out=out[blk], in_=acc[:, blk]).then_inc(s_out, 16)
                sy.wait_ge(s_out, 16 * nblk)

    return nc


def _build_elite_module():
    """Per core (64 elite rows folded to [128, D/2] bf16): e0, n0..n2 ->
    out [3, 128] = per-half-row sums of (e0 + n_t)^2."""
    import concourse.bass as bass
    from concourse import mybir

    f32 = mybir.dt.float32
    bf16 = mybir.dt.bfloat16
    HW = D // 2
    nc = bass.Bass()
    e0 = nc.declare_dram_parameter("e0", [128, HW], bf16, isOutput=False)
    nts = [
        nc.declare_dram_parameter(f"n{t}", [128, HW], bf16, isOutput=False)
        for t in range(3)
    ]
    out = nc.declare_dram_parameter("out", [3, 128], f32, isOutput=True)
    with ExitStack() as ctx:
        te = ctx.enter_context(nc.sbuf_tensor("te", [128, HW], bf16))
        tn = [
            ctx.enter_context(nc.sbuf_tensor(f"tn{t}", [128, HW], bf16))
            for t in range(3)
        ]
        ts = [
            ctx.enter_context(nc.sbuf_tensor(f"ts{t}", [128, HW], f32))
            for t in range(3)
        ]
        acc = ctx.enter_context(nc.sbuf_tensor("acc", [128, 3], f32))
        with (
            nc.Block() as block,
            nc.semaphore("s_load") as s_load,
            nc.semaphore("s_dve") as s_dve,
            nc.semaphore("s_act") as s_act,
            nc.semaphore("s_out") as s_out,
        ):

            @block.gpsimd
            def _(g):
                g.dma_start(out=te[:], in_=e0[:, :]).then_inc(s_load, 16)
                for t in range(3):
                    g.dma_start(out=tn[t][:], in_=nts[t][:, :]).then_inc(s_load, 16)

            @block.vector
            def _(v):
                for t in range(3):
                    v.wait_ge(s_load, 16 * (t + 2))
                    v.tensor_tensor(
                        out=ts[t][:], in0=te[:], in1=tn[t][:],
                        op=mybir.AluOpType.add,
                    ).then_inc(s_dve, 1)

            @block.scalar
            def _(sc):
                for t in range(3):
                    sc.wait_ge(s_dve, t + 1)
                    sc.activation(
                        out=ts[t][:], in_=ts[t][:],
                        func=mybir.ActivationFunctionType.Square,
                        accum_out=acc[:, t:t + 1],
                    ).then_inc(s_act, 1)

            @block.sync
            def _(sy):
                for t in range(3):
                    sy.wait_ge(s_act, t + 1)
                    sy.dma_start(out=out[t], in_=acc[:, t]).then_inc(s_out, 16)
                sy.wait_ge(s_out, 48)

    return nc


def _row_sumsq64(m):
    return np.einsum("ij,ij->i", m, m, dtype=np.float64)


def _init_state():
    global _state
    if _state is not None:
        return _state
    import jax
    import jax.numpy as jnp

    cpu = jax.local_devices(backend="cpu")[0]
    with jax.default_device(cpu):
        key = jax.random.key(42)
        k_scout, k_elite, k_sel, k_onl = jax.random.split(key, 4)
        ns = np.asarray(jax.random.normal(k_scout, (S, D), dtype=jnp.float32) * R)
        ne = [
            np.asarray(
                jax.random.normal(jax.random.fold_in(k_elite, t), (E, D),
                                  dtype=jnp.float32) * (R * 0.5))
            for t in range(3)
        ]
        gum = np.asarray(jax.random.gumbel(k_sel, (O, E), dtype=jnp.float32))
        no = np.asarray(jax.random.normal(k_onl, (O, D), dtype=jnp.float32) * (R * 0.3))

    st = {}
    st["ns"], st["ne"], st["gum"], st["no"] = ns, ne, gum, no
    st["ns16"] = ns.astype(BF16)
    st["ne16"] = [m.astype(BF16) for m in ne]
    st["no16"] = no.astype(BF16)
    # fp64 constants over the bf16-represented noise (exact identities)
    ne16_64 = [m.astype(np.float64) for m in st["ne16"]]
    st["nn_e16"] = [np.einsum("ij,ij->i", m, m) for m in ne16_64]
    st["nd16"] = {
        (u, v): np.einsum("ij,ij->i", ne16_64[u], ne16_64[v])
        for u in range(3) for v in range(u + 1, 3)
    }
    del ne16_64

    st["nc_scout"] = _build_fit2_module(SPC)
    st["nc_elite"] = _build_elite_module()
    st["nc_onl"] = _build_fit2_module(OPC)
    _state = st
    return st


def _run(nc, in_maps):
    from concourse.bass_utils import run_bass_kernel_spmd

    trace = bool(int(os.environ.get("BEE_TRACE", "0")))
    res = run_bass_kernel_spmd(nc, in_maps, list(range(NCORES)), trace=trace)
    _last_results.append(res)
    return res.results


def kernel(x, scout_positions, elite_positions, onlooker_positions,
           best_position, best_fitness):
    del elite_positions, onlooker_positions  # overwritten before use in reference
    st = _init_state()
    _last_results.clear()
    sp = np.ascontiguousarray(np.asarray(scout_positions, dtype=np.float32))
    ns, ne, gum, no = st["ns"], st["ne"], st["gum"], st["no"]

    # ---- phase 1: scout fitness (bf16 streams) ----
    sp16 = sp.astype(BF16)
    in1 = [
        {"a": sp16[c * SPC:(c + 1) * SPC], "b": st["ns16"][c * SPC:(c + 1) * SPC]}
        for c in range(NCORES)
    ]
    r1 = _run(st["nc_scout"], in1)
    scout_fit2 = (
        np.stack([r["out"] for r in r1]).astype(np.float64).reshape(S)
    )

    # ---- top-k elites (ascending fitness; order ties irrelevant to output) ----
    order = np.argsort(scout_fit2, kind="stable")
    idx = order[:E]
    elite0 = sp[idx] + ns[idx]  # [E, D] fp32, bit-exact vs reference

    # ---- phase 2: ||e0 + n_t||^2 for the 3 local-search steps ----
    e016 = elite0.astype(BF16)
    esq16 = _row_sumsq64(e016.astype(np.float64))  # [E]
    in2 = []
    for c in range(NCORES):
        rs = slice(c * EPC, (c + 1) * EPC)
        m = {"e0": np.ascontiguousarray(e016[rs]).reshape(128, D // 2)}
        for t in range(3):
            m[f"n{t}"] = st["ne16"][t][rs].reshape(128, D // 2)
        in2.append(m)
    r2 = _run(st["nc_elite"], in2)
    out2 = np.stack([r["out"] for r in r2]).astype(np.float64)  # [8, 3, 128]
    sq = out2.reshape(NCORES, 3, EPC, 2).sum(-1).transpose(1, 0, 2).reshape(3, E)

    # resolve the greedy steps in fp64 (accept margins ~15 in fit^2 units);
    # exact-rescue any row whose margin is within RESCUE of the boundary
    RESCUE = 1.0
    ed = [(sq[t] - esq16 - st["nn_e16"][t]) * 0.5 for t in range(3)]  # e0.n_t
    cur2 = esq16.copy()
    inset = np.zeros((E, 3), dtype=bool)
    for t in range(3):
        cs = inset.copy()
        cs[:, t] = True
        cand2 = esq16.copy()
        for u in range(3):
            cand2 = np.where(cs[:, u], cand2 + 2.0 * ed[u] + st["nn_e16"][u], cand2)
        for (u, v), dd in st["nd16"].items():
            cand2 = np.where(cs[:, u] & cs[:, v], cand2 + 2.0 * dd, cand2)
        close = np.abs(cand2 - cur2) < RESCUE
        if close.any():
            # exact fp32-vector fp64-sum recompute for borderline rows
            for j in np.nonzero(close)[0]:
                base = elite0[j].copy()
                for u in range(3):
                    if inset[j, u]:
                        base = base + ne[u][j]
                cand_row = base + ne[t][j]
                cand2[j] = float(np.einsum("i,i->", cand_row, cand_row,
                                           dtype=np.float64))
                cur2[j] = float(np.einsum("i,i->", base, base, dtype=np.float64))
        better = cand2 < cur2
        inset[:, t] |= better
        cur2 = np.where(better, cand2, cur2)
    elite_fit2 = cur2  # [E] fp64

    elite_final = elite0
    if inset.any():
        elite_final = elite0.copy()
        for t in range(3):
            rows = np.nonzero(inset[:, t])[0]
            elite_final[rows] = elite_final[rows] + ne[t][rows]

    # ---- categorical selection (replicates jax.random.categorical) ----
    elite_fit32 = np.sqrt(elite_fit2).astype(np.float32)
    sel = np.argmax(gum + (-elite_fit32)[None, :], axis=1)  # [O]

    # ---- phase 3: onlooker candidate fitness (bf16 streams) ----
    elite_final16 = elite_final.astype(BF16) if inset.any() else e016
    el_rows16 = elite_final16[sel]  # [O, D] bf16 gather
    in3 = [
        {"a": np.ascontiguousarray(el_rows16[c * OPC:(c + 1) * OPC]),
         "b": st["no16"][c * OPC:(c + 1) * OPC]}
        for c in range(NCORES)
    ]
    r3 = _run(st["nc_onl"], in3)
    onl_fit2 = np.stack([r["out"] for r in r3]).astype(np.float64).reshape(O)
    onl_better = onl_fit2 < elite_fit2[sel]
    close = np.abs(onl_fit2 - elite_fit2[sel]) < RESCUE
    for o in np.nonzero(close)[0]:
        base = elite_final[sel[o]]
        cand_row = base + no[o]
        c2 = float(np.einsum("i,i->", cand_row, cand_row, dtype=np.float64))
        b2 = float(np.einsum("i,i->", base, base, dtype=np.float64))
        onl_fit2[o] = c2
        onl_better[o] = c2 < b2

    # ---- global argmin with exact top-K rescue ----
    all_fit2 = np.concatenate([scout_fit2, elite_fit2, onl_fit2])
    K = 128
    cand_idx = np.argpartition(all_fit2, K)[:K + 1]
    cand_idx.sort()  # concat order -> first-occurrence tie semantics

    def exact_fit2(gi):
        if gi < S:
            row = sp[gi] + ns[gi]
        elif gi < S + E:
            j = gi - S
            row = elite0[j].copy()
            for t in range(3):
                if inset[j, t]:
                    row = row + ne[t][j]
        else:
            o = gi - S - E
            row = elite_final[sel[o]] + no[o]  # candidate norm is stored
        return float(np.einsum("i,i->", row, row, dtype=np.float64))

    exact = np.array([exact_fit2(int(g)) for g in cand_idx])
    bi = int(cand_idx[int(np.argmin(exact))])
    best_fit = float(np.sqrt(exact.min()))

    improved = best_fit < float(np.asarray(best_fitness))
    if not improved:
        best = np.asarray(best_position, dtype=np.float32)
    elif bi < S:
        best = sp[bi] + ns[bi]
    elif bi < S + E:
        j = bi - S
        best = elite0[j]
        for t in range(3):
            if inset[j, t]:
                best = best + ne[t][j]
    else:
        o = bi - S - E
        base = elite_final[sel[o]]
        best = (base + no[o]) if onl_better[o] else base

    batch = int(np.asarray(x).shape[0])
    return np.broadcast_to(best.astype(np.float32), (batch, D)).copy()


# revision 8
# speedup vs baseline: 1.1338x; 1.1338x over previous
"""Bee Algorithm kernel for Trainium2 (8 NeuronCores, SPMD, raw Bass).

The reference is a fixed-seed (key 42) bee algorithm: every random noise
tensor is an input-independent constant, generated once on host with the
exact jax.random calls (and ambient PRNG config) the reference uses.
The heavy, memory-bound work — per-row L2 fitness over [2048,8192]
scouts, [512,8192] elites x 3 local-search steps, and [4096,8192]
onlookers — runs on the 8 NeuronCores as streaming kernels sharded by
rows (data-parallel over the bee axis, per the sharding hint):

    per row block [128, D] (bf16 streams):
        DMA a, DMA b -> DVE add -> fp32 scratch
        -> ACT Square in place, accum_out = row fit^2 -> DMA out fits

Inputs stream in bf16: every discrete decision in the algorithm (greedy
accepts ~0.6, onlooker accepts ~0.13, global argmin gap ~1.5e-2, all in
norm units) has margins 15-700x larger than the ~8e-4 fitness error bf16
streams introduce, and the final argmin is additionally re-checked on
host with exact fp64 arithmetic over a top-K candidate set, as are any
accept decisions whose device margin is small.  The winning row itself
is reconstructed bit-exactly on host from the fp32 inputs, so the output
matches the reference to the bit whenever the (hugely separated)
decisions agree.

Phase 2 measures ||e0 + n_t||^2 for the 3 steps; the greedy recursion is
resolved on host in fp64 via exact polarization identities over the
bf16-represented vectors (noise-noise terms precomputed).
"""

import os
import sys
from contextlib import ExitStack

import numpy as np

for _p in ("/opt/trn_rl_repo", "/opt/pypackages"):
    if _p not in sys.path:
        sys.path.append(_p)

import ml_dtypes

BF16 = ml_dtypes.bfloat16

S, E, O, D = 2048, 512, 4096, 8192
R = 0.1
NCORES = 8
SPC = S // NCORES  # 256 scout rows per core
EPC = E // NCORES  # 64 elite rows per core
OPC = O // NCORES  # 512 onlooker rows per core

_state = None
_last_results = []


def _build_fit2_module(rows, width=D, nbuf=3, nscr=2):
    """Per core: a,b [rows, width] bf16 -> out [rows//128, 128] f32 row
    sums of (a+b)^2.  Block-serialized DMA issue; DVE adds into fp32
    scratch; ACT squares in place with accum_out."""
    import concourse.bass as bass
    from concourse import mybir

    f32 = mybir.dt.float32
    bf16 = mybir.dt.bfloat16
    nblk = rows // 128
    nbuf = min(nbuf, nblk)
    nscr = min(nscr, nblk)
    nc = bass.Bass()
    a = nc.declare_dram_parameter("a", [rows, width], bf16, isOutput=False)
    b = nc.declare_dram_parameter("b", [rows, width], bf16, isOutput=False)
    out = nc.declare_dram_parameter("out", [nblk, 128], f32, isOutput=True)
    with ExitStack() as ctx:
        ta = [
            ctx.enter_context(nc.sbuf_tensor(f"ta{i}", [128, width], bf16))
            for i in range(nbuf)
        ]
        tb = [
            ctx.enter_context(nc.sbuf_tensor(f"tb{i}", [128, width], bf16))
            for i in range(nbuf)
        ]
        ts = [
            ctx.enter_context(nc.sbuf_tensor(f"ts{i}", [128, width], f32))
            for i in range(nscr)
        ]
        acc = ctx.enter_context(nc.sbuf_tensor("acc", [128, nblk], f32))
        with (
            nc.Block() as block,
            nc.semaphore("s_load") as s_load,
            nc.semaphore("s_dve") as s_dve,
            nc.semaphore("s_act") as s_act,
            nc.semaphore("s_out") as s_out,
        ):

            @block.gpsimd
            def _(g):
                for blk in range(nblk):
                    i = blk % nbuf
                    if blk >= 2:
                        # allow two blocks of loads in flight: keeps the
                        # DMA engines fed while bounding reordering, and
                        # overlaps descriptor generation with transfers
                        g.wait_ge(s_load, 32 * (blk - 1))
                    if blk >= nbuf:
                        g.wait_ge(s_dve, blk - nbuf + 1)
                    g.dma_start(
                        out=ta[i][:], in_=a[blk * 128:(blk + 1) * 128, :]
                    ).then_inc(s_load, 16)
                    g.dma_start(
                        out=tb[i][:], in_=b[blk * 128:(blk + 1) * 128, :]
                    ).then_inc(s_load, 16)

            @block.vector
            def _(v):
                for blk in range(nblk):
                    i = blk % nbuf
                    j = blk % nscr
                    v.wait_ge(s_load, 32 * (blk + 1))
                    if blk >= nscr:
                        v.wait_ge(s_act, blk - nscr + 1)
                    v.tensor_tensor(
                        out=ts[j][:], in0=ta[i][:], in1=tb[i][:],
                        op=mybir.AluOpType.add,
                    ).then_inc(s_dve, 1)

            @block.scalar
            def _(sc):
                for blk in range(nblk):
                    j = blk % nscr
                    sc.wait_ge(s_dve, blk + 1)
                    sc.activation(
                        out=ts[j][:], in_=ts[j][:],
                        func=mybir.ActivationFunctionType.Square,
                        accum_out=acc[:, blk:blk + 1],
                    ).then_inc(s_act, 1)

            @block.sync
            def _(sy):
                for blk in range(nblk):
                    sy.wait_ge(s_act, blk + 1)
                    sy.dma_start(out=out[blk], in_=acc[:, blk]).then_inc(s_out, 16)
                sy.wait_ge(s_out, 16 * nblk)

    return nc


def _build_elite_module():
    """Per core (64 elite rows folded to [128, D/2] bf16): e0, n0..n2 ->
    out [3, 128] = per-half-row sums of (e0 + n_t)^2."""
    import concourse.bass as bass
    from concourse import mybir

    f32 = mybir.dt.float32
    bf16 = mybir.dt.bfloat16
    HW = D // 2
    nc = bass.Bass()
    e0 = nc.declare_dram_parameter("e0", [128, HW], bf16, isOutput=False)
    nts = [
        nc.declare_dram_parameter(f"n{t}", [128, HW], bf16, isOutput=False)
        for t in range(3)
    ]
    out = nc.declare_dram_parameter("out", [3, 128], f32, isOutput=True)
    with ExitStack() as ctx:
        te = ctx.enter_context(nc.sbuf_tensor("te", [128, HW], bf16))
        tn = [
            ctx.enter_context(nc.sbuf_tensor(f"tn{t}", [128, HW], bf16))
            for t in range(3)
        ]
        ts = [
            ctx.enter_context(nc.sbuf_tensor(f"ts{t}", [128, HW], f32))
            for t in range(3)
        ]
        acc = ctx.enter_context(nc.sbuf_tensor("acc", [128, 3], f32))
        with (
            nc.Block() as block,
            nc.semaphore("s_load") as s_load,
            nc.semaphore("s_dve") as s_dve,
            nc.semaphore("s_act") as s_act,
            nc.semaphore("s_out") as s_out,
        ):

            @block.gpsimd
            def _(g):
                g.dma_start(out=te[:], in_=e0[:, :]).then_inc(s_load, 16)
                for t in range(3):
                    g.dma_start(out=tn[t][:], in_=nts[t][:, :]).then_inc(s_load, 16)

            @block.vector
            def _(v):
                for t in range(3):
                    v.wait_ge(s_load, 16 * (t + 2))
                    v.tensor_tensor(
                        out=ts[t][:], in0=te[:], in1=tn[t][:],
                        op=mybir.AluOpType.add,
                    ).then_inc(s_dve, 1)

            @block.scalar
            def _(sc):
                for t in range(3):
                    sc.wait_ge(s_dve, t + 1)
                    sc.activation(
                        out=ts[t][:], in_=ts[t][:],
                        func=mybir.ActivationFunctionType.Square,
                        accum_out=acc[:, t:t + 1],
                    ).then_inc(s_act, 1)

            @block.sync
            def _(sy):
                for t in range(3):
                    sy.wait_ge(s_act, t + 1)
                    sy.dma_start(out=out[t], in_=acc[:, t]).then_inc(s_out, 16)
                sy.wait_ge(s_out, 48)

    return nc


def _row_sumsq64(m):
    return np.einsum("ij,ij->i", m, m, dtype=np.float64)


def _init_state():
    global _state
    if _state is not None:
        return _state
    import jax
    import jax.numpy as jnp

    cpu = jax.local_devices(backend="cpu")[0]
    with jax.default_device(cpu):
        key = jax.random.key(42)
        k_scout, k_elite, k_sel, k_onl = jax.random.split(key, 4)
        ns = np.asarray(jax.random.normal(k_scout, (S, D), dtype=jnp.float32) * R)
        ne = [
            np.asarray(
                jax.random.normal(jax.random.fold_in(k_elite, t), (E, D),
                                  dtype=jnp.float32) * (R * 0.5))
            for t in range(3)
        ]
        gum = np.asarray(jax.random.gumbel(k_sel, (O, E), dtype=jnp.float32))
        no = np.asarray(jax.random.normal(k_onl, (O, D), dtype=jnp.float32) * (R * 0.3))

    st = {}
    st["ns"], st["ne"], st["gum"], st["no"] = ns, ne, gum, no
    st["ns16"] = ns.astype(BF16)
    st["ne16"] = [m.astype(BF16) for m in ne]
    st["no16"] = no.astype(BF16)
    # fp64 constants over the bf16-represented noise (exact identities)
    ne16_64 = [m.astype(np.float64) for m in st["ne16"]]
    st["nn_e16"] = [np.einsum("ij,ij->i", m, m) for m in ne16_64]
    st["nd16"] = {
        (u, v): np.einsum("ij,ij->i", ne16_64[u], ne16_64[v])
        for u in range(3) for v in range(u + 1, 3)
    }
    del ne16_64

    st["nc_scout"] = _build_fit2_module(SPC)
    st["nc_elite"] = _build_elite_module()
    st["nc_onl"] = _build_fit2_module(OPC)
    _state = st
    return st


def _run(nc, in_maps):
    from concourse.bass_utils import run_bass_kernel_spmd

    trace = bool(int(os.environ.get("BEE_TRACE", "0")))
    res = run_bass_kernel_spmd(nc, in_maps, list(range(NCORES)), trace=trace)
    _last_results.append(res)
    return res.results


def kernel(x, scout_positions, elite_positions, onlooker_positions,
           best_position, best_fitness):
    del elite_positions, onlooker_positions  # overwritten before use in reference
    st = _init_state()
    _last_results.clear()
    sp = np.ascontiguousarray(np.asarray(scout_positions, dtype=np.float32))
    ns, ne, gum, no = st["ns"], st["ne"], st["gum"], st["no"]

    # ---- phase 1: scout fitness (bf16 streams) ----
    sp16 = sp.astype(BF16)
    in1 = [
        {"a": sp16[c * SPC:(c + 1) * SPC], "b": st["ns16"][c * SPC:(c + 1) * SPC]}
        for c in range(NCORES)
    ]
    r1 = _run(st["nc_scout"], in1)
    scout_fit2 = (
        np.stack([r["out"] for r in r1]).astype(np.float64).reshape(S)
    )

    # ---- top-k elites (ascending fitness; order ties irrelevant to output) ----
    order = np.argsort(scout_fit2, kind="stable")
    idx = order[:E]
    elite0 = sp[idx] + ns[idx]  # [E, D] fp32, bit-exact vs reference

    # ---- phase 2: ||e0 + n_t||^2 for the 3 local-search steps ----
    e016 = elite0.astype(BF16)
    esq16 = _row_sumsq64(e016.astype(np.float64))  # [E]
    in2 = []
    for c in range(NCORES):
        rs = slice(c * EPC, (c + 1) * EPC)
        m = {"e0": np.ascontiguousarray(e016[rs]).reshape(128, D // 2)}
        for t in range(3):
            m[f"n{t}"] = st["ne16"][t][rs].reshape(128, D // 2)
        in2.append(m)
    r2 = _run(st["nc_elite"], in2)
    out2 = np.stack([r["out"] for r in r2]).astype(np.float64)  # [8, 3, 128]
    sq = out2.reshape(NCORES, 3, EPC, 2).sum(-1).transpose(1, 0, 2).reshape(3, E)

    # resolve the greedy steps in fp64 (accept margins ~15 in fit^2 units);
    # exact-rescue any row whose margin is within RESCUE of the boundary
    RESCUE = 1.0
    ed = [(sq[t] - esq16 - st["nn_e16"][t]) * 0.5 for t in range(3)]  # e0.n_t
    cur2 = esq16.copy()
    inset = np.zeros((E, 3), dtype=bool)
    for t in range(3):
        cs = inset.copy()
        cs[:, t] = True
        cand2 = esq16.copy()
        for u in range(3):
            cand2 = np.where(cs[:, u], cand2 + 2.0 * ed[u] + st["nn_e16"][u], cand2)
        for (u, v), dd in st["nd16"].items():
            cand2 = np.where(cs[:, u] & cs[:, v], cand2 + 2.0 * dd, cand2)
        close = np.abs(cand2 - cur2) < RESCUE
        if close.any():
            # exact fp32-vector fp64-sum recompute for borderline rows
            for j in np.nonzero(close)[0]:
                base = elite0[j].copy()
                for u in range(3):
                    if inset[j, u]:
                        base = base + ne[u][j]
                cand_row = base + ne[t][j]
                cand2[j] = float(np.einsum("i,i->", cand_row, cand_row,
                                           dtype=np.float64))
                cur2[j] = float(np.einsum("i,i->", base, base, dtype=np.float64))
        better = cand2 < cur2
        inset[:, t] |= better
        cur2 = np.where(better, cand2, cur2)
    elite_fit2 = cur2  # [E] fp64

    elite_final = elite0
    if inset.any():
        elite_final = elite0.copy()
        for t in range(3):
            rows = np.nonzero(inset[:, t])[0]
            elite_final[rows] = elite_final[rows] + ne[t][rows]

    # ---- categorical selection (replicates jax.random.categorical) ----
    elite_fit32 = np.sqrt(elite_fit2).astype(np.float32)
    sel = np.argmax(gum + (-elite_fit32)[None, :], axis=1)  # [O]

    # ---- phase 3: onlooker candidate fitness (bf16 streams) ----
    elite_final16 = elite_final.astype(BF16) if inset.any() else e016
    el_rows16 = elite_final16[sel]  # [O, D] bf16 gather
    in3 = [
        {"a": np.ascontiguousarray(el_rows16[c * OPC:(c + 1) * OPC]),
         "b": st["no16"][c * OPC:(c + 1) * OPC]}
        for c in range(NCORES)
    ]
    r3 = _run(st["nc_onl"], in3)
    onl_fit2 = np.stack([r["out"] for r in r3]).astype(np.float64).reshape(O)
    onl_better = onl_fit2 < elite_fit2[sel]
    close = np.abs(onl_fit2 - elite_fit2[sel]) < RESCUE
    for o in np.nonzero(close)[0]:
        base = elite_final[sel[o]]
        cand_row = base + no[o]
        c2 = float(np.einsum("i,i->", cand_row, cand_row, dtype=np.float64))
        b2 = float(np.einsum("i,i->", base, base, dtype=np.float64))
        onl_fit2[o] = c2
        onl_better[o] = c2 < b2

    # ---- global argmin with exact top-K rescue ----
    all_fit2 = np.concatenate([scout_fit2, elite_fit2, onl_fit2])
    K = 128
    cand_idx = np.argpartition(all_fit2, K)[:K + 1]
    cand_idx.sort()  # concat order -> first-occurrence tie semantics

    def exact_fit2(gi):
        if gi < S:
            row = sp[gi] + ns[gi]
        elif gi < S + E:
            j = gi - S
            row = elite0[j].copy()
            for t in range(3):
                if inset[j, t]:
                    row = row + ne[t][j]
        else:
            o = gi - S - E
            row = elite_final[sel[o]] + no[o]  # candidate norm is stored
        return float(np.einsum("i,i->", row, row, dtype=np.float64))

    exact = np.array([exact_fit2(int(g)) for g in cand_idx])
    bi = int(cand_idx[int(np.argmin(exact))])
    best_fit = float(np.sqrt(exact.min()))

    improved = best_fit < float(np.asarray(best_fitness))
    if not improved:
        best = np.asarray(best_position, dtype=np.float32)
    elif bi < S:
        best = sp[bi] + ns[bi]
    elif bi < S + E:
        j = bi - S
        best = elite0[j]
        for t in range(3):
            if inset[j, t]:
                best = best + ne[t][j]
    else:
        o = bi - S - E
        base = elite_final[sel[o]]
        best = (base + no[o]) if onl_better[o] else base

    batch = int(np.asarray(x).shape[0])
    return np.broadcast_to(best.astype(np.float32), (batch, D)).copy()
